# revision 15
# baseline (speedup 1.0000x reference)
"""Trainium2 Bass kernel for EquivariantSubSampling.

The reference module reduces to a per-batch gather (verified numerically):
with (oh, ow, r) = p[b] (each in {0,1}), ic = 2*oc + r:
    r=0: out[b, oc, a, c] = x[b, ic, oh + 2a, ow + 2c]
    r=1: out[b, oc, a, c] = x[b, ic, oh + 2*((32-c) % 32), ow + 2a]

Strategy: pure data parallel over the batch dim (16 batches / 8 cores = 2
per core).  Raw flat bacc program (no Block / no end barrier).  Key points:
  - p-derived scalars arrive as a host-marshalled int32 input q
    [ohF, rF, ohL, rL, owF, owL, 1-rF, 1-rL]; engines register-load just
    what they need straight from HBM, (oh, r) pairs first so the input
    DMAs issue as early as possible
  - input rows x[b, r::2, oh::2, :] are loaded with register-offset DMAs;
    the two HWDGE rings (sync=lo halves, scalar=hi halves) each carry
    batch F first, so F's data lands ~mid-stream; F's output is enqueued
    on the sync ring behind L's input, so ring-FIFO order drains it right
    at the input tail without SWDGE traffic competing for the SDMA pool
  - batch L's hi half is split in two so only ~0.5us of gather-copies
    remain after the final input chunk lands
  - both gather variants are computed unconditionally: the r=0 variant is
    written to V[:, ds(r)] and the r=1 variant to V[:, ds(1-r)], so slot 0
    always holds the SELECTED variant and the output DMAs are fully
    static (no dynamic-AP setup on the post-copy critical path).  V is
    bf16 (halves output DMA bytes; max rel err ~0.4% << the 2e-2 gate);
    the host converts back to f32
  - live semaphores are padded into [207, 255] (the slice the NEFF
    teardown has the Sync engine clear) so Sync is the single end-of-run
    observer and the other engines enter the teardown immediately
  - no end-of-kernel barrier/cleanup: the NEFF epilogue zeroes every
    semaphore anyway; a dma_reset at kernel START (gpsimd, gating the
    first DMA issues) keeps the NEFF re-executable

Gather geometry per batch (A = SBUF copy of the 32 needed rows):
  V0[a, c] = A[a, ow + 2c]                      (r=0 variant)
  V1[a, c] = A[(32 - c) % 32, ow + 2a]          (r=1 variant)
  stage A (rows 0:16):   v0[0:16]   + v1 c {0} u [17,32)
  stage B1 (rows 16:24): v0[16:24]  + v1 c [9,17)
  stage B2 (rows 24:32): v0[24:32]  + v1 c [1,9)
"""

import numpy as np

B, C, H, W = 16, 256, 64, 64
NCORES = 8
BPC = B // NCORES           # batches per core
OC, OHW = 128, 32           # output channels, output spatial
F, L = 0, 1                 # first (hidden) / last (tail) batch slot

_COMPILED = {}


def build_nc(enable_asserts=False):
    from contextlib import ExitStack

    import concourse.bacc as bacc
    import concourse.bass as bass
    import concourse.mybir as mybir

    ds = bass.ds
    f32 = mybir.dt.float32
    bf16 = mybir.dt.bfloat16
    i32 = mybir.dt.int32
    ET = mybir.EngineType

    nc = bacc.Bacc(
        "TRN2",
        target_bir_lowering=False,
        debug=False,
        enable_asserts=enable_asserts,
        num_devices=NCORES,
    )
    x_d = nc.dram_tensor("x", [BPC, C, H, W], f32, kind="ExternalInput").ap()
    # q = host-marshalled p: [ohF, rF, ohL, rL, owF, owL, 1-rF, 1-rL]
    q_d = nc.dram_tensor("q", [1, 8], i32, kind="ExternalInput").ap()
    o_d = nc.dram_tensor("out", [BPC, OC, OHW, OHW], bf16, kind="ExternalOutput").ap()

    with ExitStack() as ctx:
        e = ctx.enter_context
        a_sb = [
            e(nc.sbuf_tensor(f"a_sb{b}", [128, 32 * 64], f32)) for b in range(BPC)
        ]
        v_sb = [
            e(nc.sbuf_tensor(f"v_sb{b}", [128, 2, OHW * OHW], bf16))
            for b in range(BPC)
        ]
        # Pad semaphore numbering so every live semaphore lands in
        # [207, 255] — the range the NEFF teardown has the SYNC engine
        # clear.  Sync is also the single end-of-kernel observer, so the
        # other engines can enter the teardown as soon as their
        # instruction streams end (their clear ranges hold only dummies).
        pads = []
        while True:
            h = nc.alloc_semaphore(f"pad{len(pads)}")
            if h.num >= 207:
                s_rst = h
                break
            pads.append(h)
        s_lo = [e(nc.semaphore(name=f"s_lo{b}")) for b in range(BPC)]
        s_hiF = e(nc.semaphore(name="s_hiF"))
        s_hiLa = e(nc.semaphore(name="s_hiLa"))
        s_hiLb = e(nc.semaphore(name="s_hiLb"))
        s_c = [e(nc.semaphore(name=f"s_c{b}")) for b in range(BPC)]
        s_outF = e(nc.semaphore(name="s_outF"))
        s_outL = e(nc.semaphore(name="s_outL"))
        all_sems = [s_rst, *s_lo, s_hiF, s_hiLa, s_hiLb, *s_c, s_outF, s_outL]
        nums = sorted(s.num for s in all_sems)
        assert nums[-1] - nums[0] + 1 == len(nums), nums  # contiguous
        sem_rng = range(nums[0], nums[-1] + 1)

        a_v = [t.ap().rearrange("p (r c) -> p r c", r=32) for t in a_sb]
        v_v = [t.ap() for t in v_sb]
        # slot-selectable 4D view: [p, slot, a, c]
        vs = [t.ap().rearrange("p s (a c) -> p s a c", a=OHW) for t in v_sb]

        def load_vals(engine_type, lo, hi):
            _, vals = nc.values_load_multi_w_load_instructions(
                q_d[0:1, lo:hi],
                engines=[engine_type],
                min_val=0,
                max_val=1,
                skip_runtime_bounds_check=True,
            )
            return vals

        # copy helpers; r selects the V slot (pass r for the r=0 variant,
        # 1-r for the r=1 variant so slot 0 holds the selected variant)
        def cp_v0(eng, b, slot, a0, a1, ow):
            return eng.tensor_copy(
                vs[b][:, ds(slot, 1), a0:a1, :],
                a_v[b][:, a0:a1, ds(ow, 32, 2)].unsqueeze(1),
            )

        def _v1_src(b, c0, c1, ow):
            # v1[:, c] = A[(32-c)%32, ow+2a]: c=0 reads row 0; c in [c0,c1)
            # with c0>=1 reads rows 32-c0 down to 33-c1 (descending)
            if c0 == 0:
                assert c1 == 1
                return a_v[b][:, 0:1, ds(ow, 32, 2)]
            return a_v[b][:, 32 - c0 : 32 - c1 : -1, ds(ow, 32, 2)]

        def cp_v1(eng, b, slot, c0, c1, ow):
            return eng.tensor_copy(
                vs[b][:, ds(slot, 1), :, c0:c1],
                _v1_src(b, c0, c1, ow).transpose([0, 2, 1]).unsqueeze(1),
            )

        def cp_v1_act(b, slot, c0, c1, ow):
            return nc.scalar.copy(
                vs[b][:, ds(slot, 1), :, c0:c1],
                _v1_src(b, c0, c1, ow).transpose([0, 2, 1]).unsqueeze(1),
            )

        # ---- gpsimd: ring reset only ----
        nc.gpsimd.dma_reset(sem_rng).then_inc(s_rst, 1)

        # ---- sync: lo halves of both batches + L's output ----
        sy = load_vals(ET.SP, 0, 4)
        nc.sync.wait_ge(s_rst, 1)
        for b, (oh, r) in ((F, (sy[0], sy[1])), (L, (sy[2], sy[3]))):
            nc.sync.dma_start(
                a_v[b][:, 0:16, :],
                x_d[b][ds(r, 128, 2), ds(oh, 16, 2), :],
            ).then_inc(s_lo[b], 16)
        nc.sync.wait_ge(s_c[F], 2)
        nc.sync.dma_start(
            o_d[F].rearrange("c h w -> c (h w)").unsqueeze(1),
            v_v[F][:, 0:1, :],
        ).then_inc(s_outF, 16)
        nc.sync.wait_ge(s_c[L], 2)
        nc.sync.dma_start(
            o_d[L].rearrange("c h w -> c (h w)").unsqueeze(1),
            v_v[L][:, 0:1, :],
        ).then_inc(s_outL, 16)
        nc.sync.wait_ge(s_outF, 16)
        nc.sync.wait_ge(s_outL, 16)

        # ---- scalar: hi halves (L's split in two) + v1/v0 copies ----
        sc = load_vals(ET.Activation, 0, 8)
        nc.scalar.wait_ge(s_rst, 1)
        nc.scalar.dma_start(
            a_v[F][:, 16:32, :],
            x_d[F][ds(sc[1], 128, 2), ds(sc[0] + 32, 16, 2), :],
        ).then_inc(s_hiF, 16)
        nc.scalar.dma_start(
            a_v[L][:, 16:24, :],
            x_d[L][ds(sc[3], 128, 2), ds(sc[2] + 32, 8, 2), :],
        ).then_inc(s_hiLa, 16)
        nc.scalar.dma_start(
            a_v[L][:, 24:32, :],
            x_d[L][ds(sc[3], 128, 2), ds(sc[2] + 48, 8, 2), :],
        ).then_inc(s_hiLb, 16)
        sc_ow, sc_nr = [sc[4], sc[5]], [sc[6], sc[7]]
        sc_r = [sc[1], sc[3]]
        for b in (F, L):
            ow, nr = sc_ow[b], sc_nr[b]
            # stage A: v1 c=0 strip (row 0) + c 17:22 (rows 15..11)
            nc.scalar.wait_ge(s_lo[b], 16)
            cp_v1_act(b, nr, 0, 1, ow)
            cp_v1_act(b, nr, 17, 22, ow)
            if b == F:
                # stage B (F): v1 c 1:9 (rows 31..24)
                nc.scalar.wait_ge(s_hiF, 16)
                cp_v1_act(F, nr, 1, 9, ow).then_inc(s_c[F], 1)
            else:
                # stage B2 (L): v0 rows 24:32 (contiguous read)
                nc.scalar.wait_ge(s_hiLb, 16)
                nc.scalar.copy(
                    vs[L][:, ds(sc_r[L], 1), 24:32, :],
                    a_v[L][:, 24:32, ds(ow, 32, 2)].unsqueeze(1),
                ).then_inc(s_c[L], 1)

        # ---- vector: v0 + the rest of v1 ----
        vv = load_vals(ET.DVE, 0, 8)
        ve_r = [vv[1], vv[3]]
        ve_ow = [vv[4], vv[5]]
        ve_nr = [vv[6], vv[7]]
        for b in (F, L):
            ow, r, nr = ve_ow[b], ve_r[b], ve_nr[b]
            nc.vector.wait_ge(s_lo[b], 16)
            cp_v0(nc.vector, b, r, 0, 16, ow)
            cp_v1(nc.vector, b, nr, 22, 32, ow)
            if b == F:
                nc.vector.wait_ge(s_hiF, 16)
                cp_v0(nc.vector, F, r, 16, 32, ow)
                cp_v1(nc.vector, F, nr, 9, 17, ow).then_inc(s_c[F], 1)
            else:
                # B1: rows 16:24 -> v0[16:24] + v1 c 9:17 (rows 23..16)
                nc.vector.wait_ge(s_hiLa, 16)
                cp_v0(nc.vector, L, r, 16, 24, ow)
                cp_v1(nc.vector, L, nr, 9, 17, ow)
                # B2: v1 c 1:9 (rows 31..24)
                nc.vector.wait_ge(s_hiLb, 16)
                cp_v1(nc.vector, L, nr, 1, 9, ow).then_inc(s_c[L], 1)

    nc.compile()
    return nc


def make_in_maps(x, p):
    x = np.ascontiguousarray(x, dtype=np.float32)
    p = np.ascontiguousarray(p, dtype=np.int32)
    assert x.shape == (B, C, H, W) and p.shape == (B, 3)
    in_maps = []
    for i in range(NCORES):
        pc = p[i * BPC : (i + 1) * BPC]
        q = np.empty((1, 8), np.int32)
        for b in range(BPC):
            q[0, 2 * b] = pc[b, 0]          # oh
            q[0, 2 * b + 1] = pc[b, 2]      # r
            q[0, 4 + b] = pc[b, 1]          # ow
            q[0, 6 + b] = 1 - pc[b, 2]      # 1-r
        in_maps.append({"x": x[i * BPC : (i + 1) * BPC], "q": q})
    return in_maps


def _get_nc():
    if "nc" not in _COMPILED:
        _COMPILED["nc"] = build_nc()
    return _COMPILED["nc"]


def kernel(x: np.ndarray, p: np.ndarray) -> np.ndarray:
    from concourse.bass_utils import run_bass_kernel_spmd

    nc = _get_nc()
    res = run_bass_kernel_spmd(nc, make_in_maps(x, p), core_ids=list(range(NCORES)))
    return np.concatenate(
        [np.asarray(res.results[i]["out"]).astype(np.float32) for i in range(NCORES)],
        axis=0,
    )


# revision 18
# speedup vs baseline: 1.0489x; 1.0489x over previous
"""Trainium2 Bass kernel for EquivariantSubSampling.

The reference module reduces to a per-batch gather (verified numerically):
with (oh, ow, r) = p[b] (each in {0,1}), ic = 2*oc + r:
    r=0: out[b, oc, a, c] = x[b, ic, oh + 2a, ow + 2c]
    r=1: out[b, oc, a, c] = x[b, ic, oh + 2*((32-c) % 32), ow + 2a]

Strategy: pure data parallel over the batch dim (16 batches / 8 cores = 2
per core).  Raw flat bacc program (no Block / no end barrier).  Key points:
  - p-derived scalars arrive as a host-marshalled int32 input q
    [ohF, rF, ohL, rL, owF, owL, 1-rF, 1-rL]; engines register-load just
    what they need straight from HBM, (oh, r) pairs first so the input
    DMAs issue as early as possible
  - input rows x[b, r::2, oh::2, :] are loaded with register-offset DMAs;
    the two HWDGE rings (sync=lo halves, scalar=hi halves) each carry
    batch F first, so F's data lands ~mid-stream and its output
    (gpsimd/SWDGE) is fully hidden under the remaining input streaming
  - batch L's hi half is split in two so only ~0.5us of gather-copies
    remain after the final input chunk lands
  - both gather variants are computed unconditionally: the r=0 variant is
    written to V[:, ds(r)] and the r=1 variant to V[:, ds(1-r)], so slot 0
    always holds the SELECTED variant and the output DMAs are fully
    static (no dynamic-AP setup on the post-copy critical path).  V is
    bf16 (halves output DMA bytes; max rel err ~0.4% << the 2e-2 gate);
    the host converts back to f32
  - no end-of-kernel barrier/cleanup: the NEFF epilogue zeroes every
    semaphore anyway; a dma_reset at kernel START (gpsimd, gating the
    first DMA issues) keeps the NEFF re-executable

Gather geometry per batch (A = SBUF copy of the 32 needed rows):
  V0[a, c] = A[a, ow + 2c]                      (r=0 variant)
  V1[a, c] = A[(32 - c) % 32, ow + 2a]          (r=1 variant)
  stage A (rows 0:16):   v0[0:16]   + v1 c {0} u [17,32)
  stage B1 (rows 16:24): v0[16:24]  + v1 c [9,17)
  stage B2 (rows 24:32): v0[24:32]  + v1 c [1,9)
"""

import numpy as np

B, C, H, W = 16, 256, 64, 64
NCORES = 8
BPC = B // NCORES           # batches per core
OC, OHW = 128, 32           # output channels, output spatial
F, L = 0, 1                 # first (hidden) / last (tail) batch slot

_COMPILED = {}


def build_nc(enable_asserts=False):
    from contextlib import ExitStack

    import concourse.bacc as bacc
    import concourse.bass as bass
    import concourse.mybir as mybir

    ds = bass.ds
    f32 = mybir.dt.float32
    bf16 = mybir.dt.bfloat16
    i32 = mybir.dt.int32
    ET = mybir.EngineType

    nc = bacc.Bacc(
        "TRN2",
        target_bir_lowering=False,
        debug=False,
        enable_asserts=enable_asserts,
        num_devices=NCORES,
    )
    x_d = nc.dram_tensor("x", [BPC, C, H, W], f32, kind="ExternalInput").ap()
    # q = host-marshalled p: [ohF, rF, ohL, rL, owF, owL, 1-rF, 1-rL]
    q_d = nc.dram_tensor("q", [1, 8], i32, kind="ExternalInput").ap()
    o_d = nc.dram_tensor("out", [BPC, OC, OHW, OHW], bf16, kind="ExternalOutput").ap()

    with ExitStack() as ctx:
        e = ctx.enter_context
        a_sb = [
            e(nc.sbuf_tensor(f"a_sb{b}", [128, 32 * 64], f32)) for b in range(BPC)
        ]
        v_sb = [
            e(nc.sbuf_tensor(f"v_sb{b}", [128, 2, OHW * OHW], bf16))
            for b in range(BPC)
        ]
        # Pad semaphore numbering so every live semaphore lands in
        # [207, 255] — the range the NEFF teardown has the SYNC engine
        # clear.  Sync is also the single end-of-kernel observer, so the
        # other engines can enter the teardown as soon as their
        # instruction streams end (their clear ranges hold only dummies).
        pads = []
        while True:
            h = nc.alloc_semaphore(f"pad{len(pads)}")
            if h.num >= 207:
                s_rst = h
                break
            pads.append(h)
        s_lo = [e(nc.semaphore(name=f"s_lo{b}")) for b in range(BPC)]
        s_hiF = e(nc.semaphore(name="s_hiF"))
        s_hiLa = e(nc.semaphore(name="s_hiLa"))
        s_hiLb = e(nc.semaphore(name="s_hiLb"))
        s_c = [e(nc.semaphore(name=f"s_c{b}")) for b in range(BPC)]
        s_outF = e(nc.semaphore(name="s_outF"))
        s_outL = e(nc.semaphore(name="s_outL"))
        all_sems = [s_rst, *s_lo, s_hiF, s_hiLa, s_hiLb, *s_c, s_outF, s_outL]
        nums = sorted(s.num for s in all_sems)
        assert nums[-1] - nums[0] + 1 == len(nums), nums  # contiguous
        sem_rng = range(nums[0], nums[-1] + 1)

        a_v = [t.ap().rearrange("p (r c) -> p r c", r=32) for t in a_sb]
        v_v = [t.ap() for t in v_sb]
        # slot-selectable 4D view: [p, slot, a, c]
        vs = [t.ap().rearrange("p s (a c) -> p s a c", a=OHW) for t in v_sb]

        def load_vals(engine_type, lo, hi):
            _, vals = nc.values_load_multi_w_load_instructions(
                q_d[0:1, lo:hi],
                engines=[engine_type],
                min_val=0,
                max_val=1,
                skip_runtime_bounds_check=True,
            )
            return vals

        # copy helpers; r selects the V slot (pass r for the r=0 variant,
        # 1-r for the r=1 variant so slot 0 holds the selected variant)
        def cp_v0(eng, b, slot, a0, a1, ow):
            return eng.tensor_copy(
                vs[b][:, ds(slot, 1), a0:a1, :],
                a_v[b][:, a0:a1, ds(ow, 32, 2)].unsqueeze(1),
            )

        def _v1_src(b, c0, c1, ow):
            # v1[:, c] = A[(32-c)%32, ow+2a]: c=0 reads row 0; c in [c0,c1)
            # with c0>=1 reads rows 32-c0 down to 33-c1 (descending)
            if c0 == 0:
                assert c1 == 1
                return a_v[b][:, 0:1, ds(ow, 32, 2)]
            return a_v[b][:, 32 - c0 : 32 - c1 : -1, ds(ow, 32, 2)]

        def cp_v1(eng, b, slot, c0, c1, ow):
            return eng.tensor_copy(
                vs[b][:, ds(slot, 1), :, c0:c1],
                _v1_src(b, c0, c1, ow).transpose([0, 2, 1]).unsqueeze(1),
            )

        def cp_v1_act(b, slot, c0, c1, ow):
            return nc.scalar.copy(
                vs[b][:, ds(slot, 1), :, c0:c1],
                _v1_src(b, c0, c1, ow).transpose([0, 2, 1]).unsqueeze(1),
            )

        # ---- gpsimd: ring reset + F's output (fully static) ----
        nc.gpsimd.dma_reset(sem_rng).then_inc(s_rst, 1)
        nc.gpsimd.wait_ge(s_c[F], 2)
        nc.gpsimd.dma_start(
            o_d[F].rearrange("c h w -> c (h w)").unsqueeze(1),
            v_v[F][:, 0:1, :],
        ).then_inc(s_outF, 16)

        # ---- sync: lo halves of both batches + L's output ----
        sy = load_vals(ET.SP, 0, 4)
        nc.sync.wait_ge(s_rst, 1)
        for b, (oh, r) in ((F, (sy[0], sy[1])), (L, (sy[2], sy[3]))):
            nc.sync.dma_start(
                a_v[b][:, 0:16, :],
                x_d[b][ds(r, 128, 2), ds(oh, 16, 2), :],
            ).then_inc(s_lo[b], 16)
        nc.sync.wait_ge(s_c[L], 2)
        nc.sync.dma_start(
            o_d[L].rearrange("c h w -> c (h w)").unsqueeze(1),
            v_v[L][:, 0:1, :],
        ).then_inc(s_outL, 16)
        nc.sync.wait_ge(s_outF, 16)
        nc.sync.wait_ge(s_outL, 16)

        # ---- scalar: hi halves (L's split in two) + v1/v0 copies ----
        sc = load_vals(ET.Activation, 0, 8)
        nc.scalar.wait_ge(s_rst, 1)
        nc.scalar.dma_start(
            a_v[F][:, 16:32, :],
            x_d[F][ds(sc[1], 128, 2), ds(sc[0] + 32, 16, 2), :],
        ).then_inc(s_hiF, 16)
        nc.scalar.dma_start(
            a_v[L][:, 16:24, :],
            x_d[L][ds(sc[3], 128, 2), ds(sc[2] + 32, 8, 2), :],
        ).then_inc(s_hiLa, 16)
        nc.scalar.dma_start(
            a_v[L][:, 24:32, :],
            x_d[L][ds(sc[3], 128, 2), ds(sc[2] + 48, 8, 2), :],
        ).then_inc(s_hiLb, 16)
        sc_ow, sc_nr = [sc[4], sc[5]], [sc[6], sc[7]]
        sc_r = [sc[1], sc[3]]
        for b in (F, L):
            ow, nr = sc_ow[b], sc_nr[b]
            # stage A: v1 c=0 strip (row 0) + c 17:22 (rows 15..11)
            nc.scalar.wait_ge(s_lo[b], 16)
            cp_v1_act(b, nr, 0, 1, ow)
            cp_v1_act(b, nr, 17, 22, ow)
            if b == F:
                # stage B (F): v1 c 1:9 (rows 31..24)
                nc.scalar.wait_ge(s_hiF, 16)
                cp_v1_act(F, nr, 1, 9, ow).then_inc(s_c[F], 1)
            else:
                # stage B2 (L): v0 rows 24:32 (contiguous read)
                nc.scalar.wait_ge(s_hiLb, 16)
                nc.scalar.copy(
                    vs[L][:, ds(sc_r[L], 1), 24:32, :],
                    a_v[L][:, 24:32, ds(ow, 32, 2)].unsqueeze(1),
                ).then_inc(s_c[L], 1)

        # ---- vector: v0 + the rest of v1 ----
        vv = load_vals(ET.DVE, 0, 8)
        ve_r = [vv[1], vv[3]]
        ve_ow = [vv[4], vv[5]]
        ve_nr = [vv[6], vv[7]]
        for b in (F, L):
            ow, r, nr = ve_ow[b], ve_r[b], ve_nr[b]
            nc.vector.wait_ge(s_lo[b], 16)
            cp_v0(nc.vector, b, r, 0, 16, ow)
            cp_v1(nc.vector, b, nr, 22, 32, ow)
            if b == F:
                nc.vector.wait_ge(s_hiF, 16)
                cp_v0(nc.vector, F, r, 16, 32, ow)
                cp_v1(nc.vector, F, nr, 9, 17, ow).then_inc(s_c[F], 1)
            else:
                # B1: rows 16:24 -> v0[16:24] + v1 c 9:17 (rows 23..16)
                nc.vector.wait_ge(s_hiLa, 16)
                cp_v0(nc.vector, L, r, 16, 24, ow)
                cp_v1(nc.vector, L, nr, 9, 17, ow)
                # B2: v1 c 1:9 (rows 31..24)
                nc.vector.wait_ge(s_hiLb, 16)
                cp_v1(nc.vector, L, nr, 1, 9, ow).then_inc(s_c[L], 1)

    nc.compile()
    return nc


def make_in_maps(x, p):
    x = np.ascontiguousarray(x, dtype=np.float32)
    p = np.ascontiguousarray(p, dtype=np.int32)
    assert x.shape == (B, C, H, W) and p.shape == (B, 3)
    in_maps = []
    for i in range(NCORES):
        pc = p[i * BPC : (i + 1) * BPC]
        q = np.empty((1, 8), np.int32)
        for b in range(BPC):
            q[0, 2 * b] = pc[b, 0]          # oh
            q[0, 2 * b + 1] = pc[b, 2]      # r
            q[0, 4 + b] = pc[b, 1]          # ow
            q[0, 6 + b] = 1 - pc[b, 2]      # 1-r
        in_maps.append({"x": x[i * BPC : (i + 1) * BPC], "q": q})
    return in_maps


def _get_nc():
    if "nc" not in _COMPILED:
        _COMPILED["nc"] = build_nc()
    return _COMPILED["nc"]


def kernel(x: np.ndarray, p: np.ndarray) -> np.ndarray:
    from concourse.bass_utils import run_bass_kernel_spmd

    nc = _get_nc()
    res = run_bass_kernel_spmd(nc, make_in_maps(x, p), core_ids=list(range(NCORES)))
    return np.concatenate(
        [np.asarray(res.results[i]["out"]).astype(np.float32) for i in range(NCORES)],
        axis=0,
    )


# revision 19
# speedup vs baseline: 1.3928x; 1.3279x over previous
"""Trainium2 Bass kernel for EquivariantSubSampling.

The reference module reduces to a per-batch gather (verified numerically):
with (oh, ow, r) = p[b] (each in {0,1}), ic = 2*oc + r:
    r=0: out[b, oc, a, c] = x[b, ic, oh + 2a, ow + 2c]
    r=1: out[b, oc, a, c] = x[b, ic, oh + 2*((32-c) % 32), ow + 2a]

Strategy: pure data parallel over the batch dim (16 batches / 8 cores = 2
per core).  Raw flat bacc program (no Block / no end barrier).  Key points:
  - p-derived scalars arrive as a host-marshalled int32 input q
    [ohF, rF, ohL, rL, owF, owL, 1-rF, 1-rL]; engines register-load just
    what they need straight from HBM, (oh, r) pairs first so the input
    DMAs issue as early as possible
  - input rows x[b, r::2, oh::2, :] are loaded with register-offset DMAs;
    the two HWDGE rings (sync=lo halves, scalar=hi halves) each carry
    batch F first, so F's data lands ~mid-stream and its output
    (gpsimd/SWDGE) is fully hidden under the remaining input streaming
  - batch L's hi half is split in two so only ~0.5us of gather-copies
    remain after the final input chunk lands
  - both gather variants are computed unconditionally: the r=0 variant is
    written to V[:, ds(r)] and the r=1 variant to V[:, ds(1-r)], so slot 0
    always holds the SELECTED variant and the output DMAs are fully
    static (no dynamic-AP setup on the post-copy critical path).  V is
    bf16 (halves output DMA bytes; max rel err ~0.4% << the 2e-2 gate);
    the host converts back to f32
  - no end-of-kernel barrier/cleanup: the NEFF epilogue zeroes every
    semaphore anyway; a dma_reset at kernel START (gpsimd, gating the
    first DMA issues) keeps the NEFF re-executable

Gather geometry per batch (A = SBUF copy of the 32 needed rows):
  V0[a, c] = A[a, ow + 2c]                      (r=0 variant)
  V1[a, c] = A[(32 - c) % 32, ow + 2a]          (r=1 variant)
  stage A (rows 0:16):   v0[0:16]   + v1 c {0} u [17,32)
  stage B1 (rows 16:24): v0[16:24]  + v1 c [9,17)
  stage B2 (rows 24:32): v0[24:32]  + v1 c [1,9)
"""

import numpy as np

B, C, H, W = 16, 256, 64, 64
NCORES = 8
BPC = B // NCORES           # batches per core
OC, OHW = 128, 32           # output channels, output spatial
F, L = 0, 1                 # first (hidden) / last (tail) batch slot

_COMPILED = {}


def build_nc(enable_asserts=False):
    from contextlib import ExitStack

    import concourse.bacc as bacc
    import concourse.bass as bass
    import concourse.mybir as mybir

    ds = bass.ds
    f32 = mybir.dt.float32
    bf16 = mybir.dt.bfloat16
    i32 = mybir.dt.int32
    ET = mybir.EngineType

    nc = bacc.Bacc(
        "TRN2",
        target_bir_lowering=False,
        debug=False,
        enable_asserts=enable_asserts,
        num_devices=NCORES,
    )
    # The __init__ preamble memsets four const-register tiles this kernel
    # never references (copies use immediate bias); dropping them lets the
    # preamble barrier clear ~0.4us earlier inside the measured window.
    entry = nc.main_func.blocks[0]
    for inst in [i for i in entry.instructions
                 if isinstance(i, mybir.InstMemset)]:
        entry.instructions.remove(inst)

    x_d = nc.dram_tensor("x", [BPC, C, H, W], f32, kind="ExternalInput").ap()
    # q = host-marshalled p: [ohF, rF, ohL, rL, owF, owL, 1-rF, 1-rL]
    q_d = nc.dram_tensor("q", [1, 8], i32, kind="ExternalInput").ap()
    o_d = nc.dram_tensor("out", [BPC, OC, OHW, OHW], bf16, kind="ExternalOutput").ap()

    with ExitStack() as ctx:
        e = ctx.enter_context
        a_sb = [
            e(nc.sbuf_tensor(f"a_sb{b}", [128, 32 * 64], f32)) for b in range(BPC)
        ]
        v_sb = [
            e(nc.sbuf_tensor(f"v_sb{b}", [128, 2, OHW * OHW], bf16))
            for b in range(BPC)
        ]
        # Pad semaphore numbering so every live semaphore lands in
        # [207, 255] — the range the NEFF teardown has the SYNC engine
        # clear.  Sync is also the single end-of-kernel observer, so the
        # other engines can enter the teardown as soon as their
        # instruction streams end (their clear ranges hold only dummies).
        pads = []
        while True:
            h = nc.alloc_semaphore(f"pad{len(pads)}")
            if h.num >= 207:
                s_rst = h
                break
            pads.append(h)
        s_lo = [e(nc.semaphore(name=f"s_lo{b}")) for b in range(BPC)]
        s_hiF = e(nc.semaphore(name="s_hiF"))
        s_hiLa = e(nc.semaphore(name="s_hiLa"))
        s_hiLb = e(nc.semaphore(name="s_hiLb"))
        s_c = [e(nc.semaphore(name=f"s_c{b}")) for b in range(BPC)]
        s_outF = e(nc.semaphore(name="s_outF"))
        s_outL = e(nc.semaphore(name="s_outL"))
        all_sems = [s_rst, *s_lo, s_hiF, s_hiLa, s_hiLb, *s_c, s_outF, s_outL]
        nums = sorted(s.num for s in all_sems)
        assert nums[-1] - nums[0] + 1 == len(nums), nums  # contiguous
        sem_rng = range(nums[0], nums[-1] + 1)

        a_v = [t.ap().rearrange("p (r c) -> p r c", r=32) for t in a_sb]
        v_v = [t.ap() for t in v_sb]
        # slot-selectable 4D view: [p, slot, a, c]
        vs = [t.ap().rearrange("p s (a c) -> p s a c", a=OHW) for t in v_sb]

        def load_vals(engine_type, lo, hi):
            _, vals = nc.values_load_multi_w_load_instructions(
                q_d[0:1, lo:hi],
                engines=[engine_type],
                min_val=0,
                max_val=1,
                skip_runtime_bounds_check=True,
            )
            return vals

        # copy helpers; r selects the V slot (pass r for the r=0 variant,
        # 1-r for the r=1 variant so slot 0 holds the selected variant)
        def cp_v0(eng, b, slot, a0, a1, ow):
            return eng.tensor_copy(
                vs[b][:, ds(slot, 1), a0:a1, :],
                a_v[b][:, a0:a1, ds(ow, 32, 2)].unsqueeze(1),
            )

        def _v1_src(b, c0, c1, ow):
            # v1[:, c] = A[(32-c)%32, ow+2a]: c=0 reads row 0; c in [c0,c1)
            # with c0>=1 reads rows 32-c0 down to 33-c1 (descending)
            if c0 == 0:
                assert c1 == 1
                return a_v[b][:, 0:1, ds(ow, 32, 2)]
            return a_v[b][:, 32 - c0 : 32 - c1 : -1, ds(ow, 32, 2)]

        def cp_v1(eng, b, slot, c0, c1, ow):
            return eng.tensor_copy(
                vs[b][:, ds(slot, 1), :, c0:c1],
                _v1_src(b, c0, c1, ow).transpose([0, 2, 1]).unsqueeze(1),
            )

        def cp_v1_act(b, slot, c0, c1, ow):
            return nc.scalar.copy(
                vs[b][:, ds(slot, 1), :, c0:c1],
                _v1_src(b, c0, c1, ow).transpose([0, 2, 1]).unsqueeze(1),
            )

        # ---- gpsimd: ring reset + F's output (fully static) ----
        nc.gpsimd.dma_reset(sem_rng).then_inc(s_rst, 1)
        nc.gpsimd.wait_ge(s_c[F], 2)
        nc.gpsimd.dma_start(
            o_d[F].rearrange("c h w -> c (h w)").unsqueeze(1),
            v_v[F][:, 0:1, :],
        ).then_inc(s_outF, 16)

        # ---- sync: lo halves of both batches + L's output ----
        sy = load_vals(ET.SP, 0, 4)
        nc.sync.wait_ge(s_rst, 1)
        for b, (oh, r) in ((F, (sy[0], sy[1])), (L, (sy[2], sy[3]))):
            nc.sync.dma_start(
                a_v[b][:, 0:16, :],
                x_d[b][ds(r, 128, 2), ds(oh, 16, 2), :],
            ).then_inc(s_lo[b], 16)
        nc.sync.wait_ge(s_c[L], 2)
        nc.sync.dma_start(
            o_d[L].rearrange("c h w -> c (h w)").unsqueeze(1),
            v_v[L][:, 0:1, :],
        ).then_inc(s_outL, 16)
        nc.sync.wait_ge(s_outF, 16)
        nc.sync.wait_ge(s_outL, 16)

        # ---- scalar: hi halves (L's split in two) + v1/v0 copies ----
        sc = load_vals(ET.Activation, 0, 8)
        nc.scalar.wait_ge(s_rst, 1)
        nc.scalar.dma_start(
            a_v[F][:, 16:32, :],
            x_d[F][ds(sc[1], 128, 2), ds(sc[0] + 32, 16, 2), :],
        ).then_inc(s_hiF, 16)
        nc.scalar.dma_start(
            a_v[L][:, 16:24, :],
            x_d[L][ds(sc[3], 128, 2), ds(sc[2] + 32, 8, 2), :],
        ).then_inc(s_hiLa, 16)
        nc.scalar.dma_start(
            a_v[L][:, 24:32, :],
            x_d[L][ds(sc[3], 128, 2), ds(sc[2] + 48, 8, 2), :],
        ).then_inc(s_hiLb, 16)
        sc_ow, sc_nr = [sc[4], sc[5]], [sc[6], sc[7]]
        sc_r = [sc[1], sc[3]]
        for b in (F, L):
            ow, nr = sc_ow[b], sc_nr[b]
            # stage A: v1 c=0 strip (row 0) + c 17:22 (rows 15..11)
            nc.scalar.wait_ge(s_lo[b], 16)
            cp_v1_act(b, nr, 0, 1, ow)
            cp_v1_act(b, nr, 17, 22, ow)
            if b == F:
                # stage B (F): v1 c 1:9 (rows 31..24)
                nc.scalar.wait_ge(s_hiF, 16)
                cp_v1_act(F, nr, 1, 9, ow).then_inc(s_c[F], 1)
            else:
                # stage B2 (L): v0 rows 24:32 (contiguous read)
                nc.scalar.wait_ge(s_hiLb, 16)
                nc.scalar.copy(
                    vs[L][:, ds(sc_r[L], 1), 24:32, :],
                    a_v[L][:, 24:32, ds(ow, 32, 2)].unsqueeze(1),
                ).then_inc(s_c[L], 1)

        # ---- vector: v0 + the rest of v1 ----
        vv = load_vals(ET.DVE, 0, 8)
        ve_r = [vv[1], vv[3]]
        ve_ow = [vv[4], vv[5]]
        ve_nr = [vv[6], vv[7]]
        for b in (F, L):
            ow, r, nr = ve_ow[b], ve_r[b], ve_nr[b]
            nc.vector.wait_ge(s_lo[b], 16)
            cp_v0(nc.vector, b, r, 0, 16, ow)
            cp_v1(nc.vector, b, nr, 22, 32, ow)
            if b == F:
                nc.vector.wait_ge(s_hiF, 16)
                cp_v0(nc.vector, F, r, 16, 32, ow)
                cp_v1(nc.vector, F, nr, 9, 17, ow).then_inc(s_c[F], 1)
            else:
                # B1: rows 16:24 -> v0[16:24] + v1 c 9:17 (rows 23..16)
                nc.vector.wait_ge(s_hiLa, 16)
                cp_v0(nc.vector, L, r, 16, 24, ow)
                cp_v1(nc.vector, L, nr, 9, 17, ow)
                # B2: v1 c 1:9 (rows 31..24)
                nc.vector.wait_ge(s_hiLb, 16)
                cp_v1(nc.vector, L, nr, 1, 9, ow).then_inc(s_c[L], 1)

    nc.compile()
    return nc


def make_in_maps(x, p):
    x = np.ascontiguousarray(x, dtype=np.float32)
    p = np.ascontiguousarray(p, dtype=np.int32)
    assert x.shape == (B, C, H, W) and p.shape == (B, 3)
    in_maps = []
    for i in range(NCORES):
        pc = p[i * BPC : (i + 1) * BPC]
        q = np.empty((1, 8), np.int32)
        for b in range(BPC):
            q[0, 2 * b] = pc[b, 0]          # oh
            q[0, 2 * b + 1] = pc[b, 2]      # r
            q[0, 4 + b] = pc[b, 1]          # ow
            q[0, 6 + b] = 1 - pc[b, 2]      # 1-r
        in_maps.append({"x": x[i * BPC : (i + 1) * BPC], "q": q})
    return in_maps


def _get_nc():
    if "nc" not in _COMPILED:
        _COMPILED["nc"] = build_nc()
    return _COMPILED["nc"]


def kernel(x: np.ndarray, p: np.ndarray) -> np.ndarray:
    from concourse.bass_utils import run_bass_kernel_spmd

    nc = _get_nc()
    res = run_bass_kernel_spmd(nc, make_in_maps(x, p), core_ids=list(range(NCORES)))
    return np.concatenate(
        [np.asarray(res.results[i]["out"]).astype(np.float32) for i in range(NCORES)],
        axis=0,
    )


# revision 20
# speedup vs baseline: 1.7782x; 1.2767x over previous
"""Trainium2 Bass kernel for EquivariantSubSampling.

The reference module reduces to a per-batch gather (verified numerically):
with (oh, ow, r) = p[b] (each in {0,1}), ic = 2*oc + r:
    r=0: out[b, oc, a, c] = x[b, ic, oh + 2a, ow + 2c]
    r=1: out[b, oc, a, c] = x[b, ic, oh + 2*((32-c) % 32), ow + 2a]

Strategy: pure data parallel over the batch dim (16 batches / 8 cores = 2
per core).  Raw flat bacc program (no Block / no end barrier).  Key points:
  - p-derived scalars arrive as a host-marshalled int32 input q
    [ohF, rF, ohL, rL, owF, owL, 1-rF, 1-rL]; engines register-load just
    what they need straight from HBM, (oh, r) pairs first so the input
    DMAs issue as early as possible
  - input rows x[b, r::2, oh::2, :] are loaded with register-offset DMAs;
    the two HWDGE rings (sync=lo halves, scalar=hi halves) each carry
    batch F first, so F's data lands ~mid-stream and its output
    (gpsimd/SWDGE) is fully hidden under the remaining input streaming
  - batch L's hi half is split in two so only ~0.5us of gather-copies
    remain after the final input chunk lands
  - both gather variants are computed unconditionally: the r=0 variant is
    written to V[:, ds(r)] and the r=1 variant to V[:, ds(1-r)], so slot 0
    always holds the SELECTED variant and the output DMAs are fully
    static (no dynamic-AP setup on the post-copy critical path).  V is
    bf16 (halves output DMA bytes; max rel err ~0.4% << the 2e-2 gate);
    the host converts back to f32
  - no end-of-kernel barrier/cleanup: the NEFF epilogue zeroes every
    semaphore anyway; a dma_reset at kernel START (gpsimd, gating the
    first DMA issues) keeps the NEFF re-executable

Gather geometry per batch (A = SBUF copy of the 32 needed rows):
  V0[a, c] = A[a, ow + 2c]                      (r=0 variant)
  V1[a, c] = A[(32 - c) % 32, ow + 2a]          (r=1 variant)
  stage A (rows 0:16):   v0[0:16]   + v1 c {0} u [17,32)
  stage B1 (rows 16:24): v0[16:24]  + v1 c [9,17)
  stage B2 (rows 24:32): v0[24:32]  + v1 c [1,9)
"""

import numpy as np

B, C, H, W = 16, 256, 64, 64
NCORES = 8
BPC = B // NCORES           # batches per core
OC, OHW = 128, 32           # output channels, output spatial
F, L = 0, 1                 # first (hidden) / last (tail) batch slot

_COMPILED = {}


def build_nc(enable_asserts=False):
    from contextlib import ExitStack

    import concourse.bacc as bacc
    import concourse.bass as bass
    import concourse.mybir as mybir

    ds = bass.ds
    f32 = mybir.dt.float32
    bf16 = mybir.dt.bfloat16
    i32 = mybir.dt.int32
    ET = mybir.EngineType

    nc = bacc.Bacc(
        "TRN2",
        target_bir_lowering=False,
        debug=False,
        enable_asserts=enable_asserts,
        num_devices=NCORES,
    )
    # The __init__ preamble memsets four const-register tiles this kernel
    # never references (copies use immediate bias); dropping them lets the
    # preamble barrier clear ~0.4us earlier inside the measured window.
    entry = nc.main_func.blocks[0]
    for inst in [i for i in entry.instructions
                 if isinstance(i, mybir.InstMemset)]:
        entry.instructions.remove(inst)

    x_d = nc.dram_tensor("x", [BPC, C, H, W], f32, kind="ExternalInput").ap()
    # q = host-marshalled p: [ohF, rF, ohL, rL, owF, owL, 1-rF, 1-rL]
    q_d = nc.dram_tensor("q", [1, 8], i32, kind="ExternalInput").ap()
    o_d = nc.dram_tensor("out", [BPC, OC, OHW, OHW], bf16, kind="ExternalOutput").ap()

    with ExitStack() as ctx:
        e = ctx.enter_context
        a_sb = [
            e(nc.sbuf_tensor(f"a_sb{b}", [128, 32 * 64], f32)) for b in range(BPC)
        ]
        v_sb = [
            e(nc.sbuf_tensor(f"v_sb{b}", [128, 2, OHW * OHW], bf16))
            for b in range(BPC)
        ]
        # Pad semaphore numbering so every live semaphore lands in
        # [207, 255] — the range the NEFF teardown has the SYNC engine
        # clear.  Sync is also the single end-of-kernel observer, so the
        # other engines can enter the teardown as soon as their
        # instruction streams end (their clear ranges hold only dummies).
        pads = []
        while True:
            h = nc.alloc_semaphore(f"pad{len(pads)}")
            if h.num >= 207:
                s_rst = h
                break
            pads.append(h)
        s_lo = [e(nc.semaphore(name=f"s_lo{b}")) for b in range(BPC)]
        s_hiF = e(nc.semaphore(name="s_hiF"))
        s_hiLa = e(nc.semaphore(name="s_hiLa"))
        s_hiLb = e(nc.semaphore(name="s_hiLb"))
        s_c = [e(nc.semaphore(name=f"s_c{b}")) for b in range(BPC)]
        s_outF = e(nc.semaphore(name="s_outF"))
        s_outL = e(nc.semaphore(name="s_outL"))
        all_sems = [s_rst, *s_lo, s_hiF, s_hiLa, s_hiLb, *s_c, s_outF, s_outL]
        nums = sorted(s.num for s in all_sems)
        assert nums[-1] - nums[0] + 1 == len(nums), nums  # contiguous
        sem_rng = range(nums[0], nums[-1] + 1)

        a_v = [t.ap().rearrange("p (r c) -> p r c", r=32) for t in a_sb]
        v_v = [t.ap() for t in v_sb]
        # slot-selectable 4D view: [p, slot, a, c]
        vs = [t.ap().rearrange("p s (a c) -> p s a c", a=OHW) for t in v_sb]

        def load_vals(engine_type, lo, hi):
            _, vals = nc.values_load_multi_w_load_instructions(
                q_d[0:1, lo:hi],
                engines=[engine_type],
                min_val=0,
                max_val=1,
                skip_runtime_bounds_check=True,
            )
            return vals

        # copy helpers; r selects the V slot (pass r for the r=0 variant,
        # 1-r for the r=1 variant so slot 0 holds the selected variant)
        def cp_v0(eng, b, slot, a0, a1, ow):
            return eng.tensor_copy(
                vs[b][:, ds(slot, 1), a0:a1, :],
                a_v[b][:, a0:a1, ds(ow, 32, 2)].unsqueeze(1),
            )

        def _v1_src(b, c0, c1, ow):
            # v1[:, c] = A[(32-c)%32, ow+2a]: c=0 reads row 0; c in [c0,c1)
            # with c0>=1 reads rows 32-c0 down to 33-c1 (descending)
            if c0 == 0:
                assert c1 == 1
                return a_v[b][:, 0:1, ds(ow, 32, 2)]
            return a_v[b][:, 32 - c0 : 32 - c1 : -1, ds(ow, 32, 2)]

        def cp_v1(eng, b, slot, c0, c1, ow):
            return eng.tensor_copy(
                vs[b][:, ds(slot, 1), :, c0:c1],
                _v1_src(b, c0, c1, ow).transpose([0, 2, 1]).unsqueeze(1),
            )

        def cp_v1_act(b, slot, c0, c1, ow):
            return nc.scalar.copy(
                vs[b][:, ds(slot, 1), :, c0:c1],
                _v1_src(b, c0, c1, ow).transpose([0, 2, 1]).unsqueeze(1),
            )

        # ---- gpsimd: ring reset + F's output (fully static) ----
        nc.gpsimd.dma_reset(sem_rng).then_inc(s_rst, 1)
        nc.gpsimd.wait_ge(s_c[F], 2)
        nc.gpsimd.dma_start(
            o_d[F].rearrange("c h w -> c (h w)").unsqueeze(1),
            v_v[F][:, 0:1, :],
        ).then_inc(s_outF, 16)

        # ---- sync: lo halves of both batches + L's output ----
        sy = load_vals(ET.SP, 0, 4)
        nc.sync.wait_ge(s_rst, 1)
        for b, (oh, r) in ((F, (sy[0], sy[1])), (L, (sy[2], sy[3]))):
            nc.sync.dma_start(
                a_v[b][:, 0:16, :],
                x_d[b][ds(r, 128, 2), ds(oh, 16, 2), :],
            ).then_inc(s_lo[b], 16)
        nc.sync.wait_ge(s_c[L], 2)
        nc.sync.dma_start(
            o_d[L].rearrange("c h w -> c (h w)").unsqueeze(1),
            v_v[L][:, 0:1, :],
        ).then_inc(s_outL, 16)
        nc.sync.wait_ge(s_outF, 16)
        nc.sync.wait_ge(s_outL, 16)

        # ---- scalar: hi halves (L's split in two) + v1/v0 copies ----
        sc = load_vals(ET.Activation, 0, 8)
        nc.scalar.wait_ge(s_rst, 1)
        nc.scalar.dma_start(
            a_v[F][:, 16:32, :],
            x_d[F][ds(sc[1], 128, 2), ds(sc[0] + 32, 16, 2), :],
        ).then_inc(s_hiF, 16)
        nc.scalar.dma_start(
            a_v[L][:, 16:24, :],
            x_d[L][ds(sc[3], 128, 2), ds(sc[2] + 32, 8, 2), :],
        ).then_inc(s_hiLa, 16)
        nc.scalar.dma_start(
            a_v[L][:, 24:32, :],
            x_d[L][ds(sc[3], 128, 2), ds(sc[2] + 48, 8, 2), :],
        ).then_inc(s_hiLb, 16)
        sc_ow, sc_nr = [sc[4], sc[5]], [sc[6], sc[7]]
        sc_r = [sc[1], sc[3]]
        for b in (F, L):
            ow, nr = sc_ow[b], sc_nr[b]
            # stage A: v1 c=0 strip (row 0) + c 17:22 (rows 15..11)
            nc.scalar.wait_ge(s_lo[b], 16)
            if b == F:
                nc.scalar.wait_ge(s_hiF, 16)
            cp_v1_act(b, nr, 0, 1, ow)
            cp_v1_act(b, nr, 17, 22, ow)
            if b == F:
                # stage B (F): v1 c 1:9 (rows 31..24)
                nc.scalar.wait_ge(s_hiF, 16)
                cp_v1_act(F, nr, 1, 9, ow).then_inc(s_c[F], 1)
            else:
                # stage B2 (L): v0 rows 24:32 (contiguous read)
                nc.scalar.wait_ge(s_hiLb, 16)
                nc.scalar.copy(
                    vs[L][:, ds(sc_r[L], 1), 24:32, :],
                    a_v[L][:, 24:32, ds(ow, 32, 2)].unsqueeze(1),
                ).then_inc(s_c[L], 1)

        # ---- vector: v0 + the rest of v1 ----
        vv = load_vals(ET.DVE, 0, 8)
        ve_r = [vv[1], vv[3]]
        ve_ow = [vv[4], vv[5]]
        ve_nr = [vv[6], vv[7]]
        for b in (F, L):
            ow, r, nr = ve_ow[b], ve_r[b], ve_nr[b]
            nc.vector.wait_ge(s_lo[b], 16)
            if b == F:
                # F's copies are only needed in time for F-out (~19us);
                # starting them at F-hi instead of F-lo keeps the whole
                # stage-A+B block contiguous and as late as harmless
                nc.vector.wait_ge(s_hiF, 16)
            cp_v0(nc.vector, b, r, 0, 16, ow)
            cp_v1(nc.vector, b, nr, 22, 32, ow)
            if b == F:
                nc.vector.wait_ge(s_hiF, 16)
                cp_v0(nc.vector, F, r, 16, 32, ow)
                cp_v1(nc.vector, F, nr, 9, 17, ow).then_inc(s_c[F], 1)
            else:
                # B1: rows 16:24 -> v0[16:24] + v1 c 9:17 (rows 23..16)
                nc.vector.wait_ge(s_hiLa, 16)
                cp_v0(nc.vector, L, r, 16, 24, ow)
                cp_v1(nc.vector, L, nr, 9, 17, ow)
                # B2: v1 c 1:9 (rows 31..24)
                nc.vector.wait_ge(s_hiLb, 16)
                cp_v1(nc.vector, L, nr, 1, 9, ow).then_inc(s_c[L], 1)

    nc.compile()
    return nc


def make_in_maps(x, p):
    x = np.ascontiguousarray(x, dtype=np.float32)
    p = np.ascontiguousarray(p, dtype=np.int32)
    assert x.shape == (B, C, H, W) and p.shape == (B, 3)
    in_maps = []
    for i in range(NCORES):
        pc = p[i * BPC : (i + 1) * BPC]
        q = np.empty((1, 8), np.int32)
        for b in range(BPC):
            q[0, 2 * b] = pc[b, 0]          # oh
            q[0, 2 * b + 1] = pc[b, 2]      # r
            q[0, 4 + b] = pc[b, 1]          # ow
            q[0, 6 + b] = 1 - pc[b, 2]      # 1-r
        in_maps.append({"x": x[i * BPC : (i + 1) * BPC], "q": q})
    return in_maps


def _get_nc():
    if "nc" not in _COMPILED:
        _COMPILED["nc"] = build_nc()
    return _COMPILED["nc"]


def kernel(x: np.ndarray, p: np.ndarray) -> np.ndarray:
    from concourse.bass_utils import run_bass_kernel_spmd

    nc = _get_nc()
    res = run_bass_kernel_spmd(nc, make_in_maps(x, p), core_ids=list(range(NCORES)))
    return np.concatenate(
        [np.asarray(res.results[i]["out"]).astype(np.float32) for i in range(NCORES)],
        axis=0,
    )
